# revision 17
# baseline (speedup 1.0000x reference)
"""Single-head self-attention (CrossVit block) on 8 Trainium2 NeuronCores.

Computation (fp32 reference):
    q = x @ Wq + bq ; k = x @ Wk + bk ; v = x @ Wv + bv        [S, E]
    scores = (q @ k^T) / sqrt(E)                               [S, S]
    out = softmax(scores, axis=-1) @ v                         [S, E]
with S = 8192, E = 2048.

Strategy (sequence-parallel over query rows, 1024 per core):
  Host: transpose x -> xT [E, S] (bf16) and hand each core its column
  slice xT_i [E, 1024] plus full Wq/Wk/Wv (bf16) and biases (fp32).
  Device, per core i:
    phase 0: KT_i = Wk^T xT_i (+bk) -> DRAM (4 separate slice tiles),
             each AllGather'd pipelined against the projection itself;
             V_i = xT_i Wv -> DRAM (2 separate slice tiles), AllGather'd;
             QT_i = Wq^T xT_i (+bq) stays in SBUF.
    phase 1: ST[j] = KT_j^T @ QT  (scores transposed: [sk, sq]) -> exp -> SBUF
             l_acc[p, sq] += exp(ST[j])   (running sums on the Vector engine)
    phase 1b: l[sq] = ones^T @ l_acc  (2 fp32 matmuls), transpose via DRAM,
             reciprocal.
    phase 2: O[sq, e] = sum_j exp(ST[j])^T @ V_j, 8 passes of 4 PSUM banks
             (eq x s-half) so epilogues overlap the next pass,
             epilogue: O * (1/l) + bv -> out
  Host: concatenate the 8 row blocks.
"""

import math

import numpy as np
import ml_dtypes

S = 8192
E = 2048
N_CORES = 8
SL = S // N_CORES      # 1024 query rows per core
P = 128                # partitions
ECH = E // P           # 16 contraction chunks
JN = S // P            # 64 global key chunks
NQ = 512               # moving free-dim (ISA max per matmul)
QCH = SL // NQ         # 2 query chunks of 512
SCH = SL // P          # 8 query chunks of 128
EQN = E // NQ          # 4 output-column chunks of 512
KAG = 4                # AllGather splits for KT (rows of 512)
VAG = 2                # AllGather splits for V (rows of 512)
EHALF = ECH // 2       # 8 contraction chunks per weight half-tile

_BF16 = ml_dtypes.bfloat16


def _build():
    import concourse.bacc as bacc
    import concourse.bass as bass
    import concourse.tile as tile
    import concourse.mybir as mybir

    bf16 = mybir.dt.bfloat16
    f32 = mybir.dt.float32
    SCALE = 1.0 / math.sqrt(float(E))

    nc = bacc.Bacc("TRN2", target_bir_lowering=False, debug=False,
                   num_devices=N_CORES)

    xt = nc.declare_dram_parameter("xt", [E, SL], bf16, isOutput=False)
    wq = nc.declare_dram_parameter("wq", [ECH, P, ECH, P], bf16, isOutput=False)
    wk = nc.declare_dram_parameter("wk", [ECH, P, ECH, P], bf16, isOutput=False)
    wv = nc.declare_dram_parameter("wv", [E, E], bf16, isOutput=False)
    bq = nc.declare_dram_parameter("bq", [E], f32, isOutput=False)
    bk = nc.declare_dram_parameter("bk", [E], f32, isOutput=False)
    bv = nc.declare_dram_parameter("bv", [E], bf16, isOutput=False)
    out = nc.declare_dram_parameter("out", [SL, E], f32, isOutput=True)

    groups = [list(range(N_CORES))]
    KSL = E // KAG        # 512 rows per KT AG slice
    VSL = SL // VAG       # 512 rows per V AG slice

    with tile.TileContext(nc) as tc:
        with (
            tc.tile_pool(name="dram", bufs=1, space="DRAM") as dram,
            tc.tile_pool(name="big", bufs=1) as big,
            tc.tile_pool(name="res", bufs=1) as res,
            tc.tile_pool(name="wstr", bufs=6) as wstr,
            tc.tile_pool(name="vstr", bufs=8) as vstr,
            tc.tile_pool(name="stg", bufs=2) as stg,
            tc.tile_pool(name="ps", bufs=8, space="PSUM") as ps,
        ):
            # separate staging tiles per AllGather slice: no write-after-read
            # hazard between projection slice i+1 and the AG of slice i.
            kt_in = [dram.tile([KSL, SL], bf16, name=f"kt_in_{i}")
                     for i in range(KAG)]
            v_in = [dram.tile([VSL, E], bf16, name=f"v_in_{h}")
                    for h in range(VAG)]
            kt_all = [dram.tile([N_CORES * KSL, SL], bf16, addr_space="Shared",
                                name=f"kt_all_{i}") for i in range(KAG)]
            v_all = [dram.tile([N_CORES * VSL, E], bf16, addr_space="Shared",
                               name=f"v_all_{i}") for i in range(VAG)]
            l_dram = dram.tile([1, SL], f32)

            # --- resident SBUF tensors -------------------------------------
            # xt_sb and st_sb share one 128KB/partition slot (disjoint
            # lifetimes: xt only in phase 0, st written in phase 1).
            xt_sb = big.tile([P, ECH, SL], bf16, tag="bigslot")
            qt_sb = res.tile([P, ECH, SL], bf16)
            bq_sb = res.tile([P, ECH], f32)
            bk_sb = res.tile([P, ECH], f32)
            ones_sb = res.tile([P, 1], f32)

            # startup: xt chunks 0-1, then the first weight tiles, then the
            # rest of xt — the first matmul can start after ~2 small DMAs.
            def _load_w(wtag, w_param, eo):
                w_h = []
                for hw in range(2):
                    t = wstr.tile([P, EHALF, P], bf16, tag="w",
                                  name=f"w_{wtag}_{eo}_{hw}")
                    nc.sync.dma_start(
                        out=t, in_=w_param[eo, :, hw * EHALF:(hw + 1) * EHALF])
                    w_h.append(t)
                return w_h

            for ec in range(ECH):
                nc.scalar.dma_start(out=xt_sb[:, ec],
                                    in_=xt[ec * P:(ec + 1) * P])
            w_pre = _load_w("wk", wk, 0)
            nc.scalar.dma_start(out=bq_sb, in_=bq.rearrange("(c p) -> p c", p=P))
            nc.scalar.dma_start(out=bk_sb, in_=bk.rearrange("(c p) -> p c", p=P))
            nc.vector.memset(ones_sb, 1.0)

            # --- phase 0a: KT_i = Wk^T @ xT_i + bk -> kt_in, AG in 4 slices -
            def qk_proj(w_param, b_sb, dst_sbuf, dst_dram, wtag, eo_lo, eo_hi,
                        w_pre=None):
                for eo in range(eo_lo, eo_hi):
                    # two half-eo weight tiles for finer prefetch granularity
                    if w_pre is not None and eo == eo_lo:
                        w_h = w_pre
                    else:
                        w_h = _load_w(wtag, w_param, eo)
                    for q in range(QCH):
                        acc = ps.tile([P, NQ], f32, tag="mm",
                                      name=f"acc_{wtag}_{eo}_{q}")
                        for ec in range(ECH):
                            nc.tensor.matmul(
                                acc, w_h[ec // EHALF][:, ec % EHALF],
                                xt_sb[:, ec, q * NQ:(q + 1) * NQ],
                                start=(ec == 0), stop=(ec == ECH - 1))
                        if dst_sbuf is not None:
                            nc.scalar.activation(
                                dst_sbuf[:, eo, q * NQ:(q + 1) * NQ], acc,
                                mybir.ActivationFunctionType.Identity,
                                bias=b_sb[:, eo:eo + 1], scale=1.0)
                        else:
                            kstg = stg.tile([P, NQ], bf16, tag="kstg",
                                            name=f"kstg_{eo}_{q}")
                            nc.scalar.activation(
                                kstg, acc,
                                mybir.ActivationFunctionType.Identity,
                                bias=b_sb[:, eo:eo + 1], scale=1.0)
                            lo = (eo * P) % KSL
                            nc.scalar.dma_start(
                                out=dst_dram[lo:lo + P,
                                             q * NQ:(q + 1) * NQ],
                                in_=kstg)

            eo_per_slice = KSL // P  # 4
            for i in range(KAG):
                qk_proj(wk, bk_sb, None, kt_in[i], "wk",
                        i * eo_per_slice, (i + 1) * eo_per_slice,
                        w_pre=w_pre if i == 0 else None)
                nc.gpsimd.collective_compute(
                    "AllGather", mybir.AluOpType.bypass, replica_groups=groups,
                    ins=[kt_in[i].opt()],
                    outs=[kt_all[i].opt()])

            # --- phase 0b: V_i = xT_i^T @ Wv -> v_in -------------------------
            # bv is folded into the epilogue (attn rows sum to 1).
            # eq-major with all 8 s-chunks per pass: wv is streamed once
            # (8 MB, not 16) at half the rate, so a deeper wv runway rides
            # out the kt-AllGather HBM bursts. The v AllGathers fire later,
            # from inside phase 1 (see below).
            for eq in range(EQN):           # e-quarters of 512
                accs = [ps.tile([P, NQ], f32, tag="mm",
                                name=f"vacc_{eq}_{s}")
                        for s in range(SCH)]
                for ec in range(ECH):
                    wv_t = vstr.tile([P, NQ], bf16, tag="wv",
                                     name=f"wv_{eq}_{ec}")
                    nc.sync.dma_start(
                        out=wv_t,
                        in_=wv[ec * P:(ec + 1) * P,
                               eq * NQ:(eq + 1) * NQ])
                    for s in range(SCH):
                        nc.tensor.matmul(
                            accs[s], xt_sb[:, ec, s * P:(s + 1) * P],
                            wv_t, start=(ec == 0), stop=(ec == ECH - 1))
                for s in range(SCH):
                    vstg = stg.tile([P, NQ], bf16, tag="vstg",
                                    name=f"vstg_{eq}_{s}")
                    nc.vector.tensor_copy(out=vstg, in_=accs[s])
                    h, lo = (s * P) // VSL, (s * P) % VSL
                    nc.scalar.dma_start(
                        out=v_in[h][lo:lo + P, eq * NQ:(eq + 1) * NQ],
                        in_=vstg)

            # --- phase 0c: QT_i = Wq^T @ xT_i + bq -> qt_sb (SBUF-resident) -
            qk_proj(wq, bq_sb, qt_sb, None, "wq", 0, ECH)

            # --- phase 1: ST[j] = KT_j^T @ QT, exp; l_acc on Vector engine --
            st_sb = big.tile([P, JN, SL], bf16, tag="bigslot")
            l_acc = res.tile([P, SL], f32, tag="lacc")
            nc.vector.memset(l_acc, 0.0)
            ec_per_slice = ECH // KAG  # 4 e-chunks per AG slice
            for j in range(JN):
                r, c = j // SCH, j % SCH
                # shares the "w" tag: its buffer slot only frees late in
                # phase 0, which pins this DMA's static queue position past
                # the AllGathers (a hoisted AG-dependent DMA head-of-line
                # blocks the whole sync queue).
                kt_t = wstr.tile([P, ECH, P], bf16, tag="w", name=f"kt_{j}")
                for i in range(KAG):
                    nc.sync.dma_start(
                        out=kt_t[:, i * ec_per_slice:(i + 1) * ec_per_slice, :],
                        in_=kt_all[i][r * KSL:(r + 1) * KSL,
                                      c * P:(c + 1) * P].rearrange(
                                          "(ec p) s -> p ec s", p=P))
                for q in range(QCH):
                    st_ps = ps.tile([P, NQ], f32, tag="mm", name=f"st_{j}_{q}")
                    for ec in range(ECH):
                        nc.tensor.matmul(
                            st_ps, kt_t[:, ec],
                            qt_sb[:, ec, q * NQ:(q + 1) * NQ],
                            start=(ec == 0), stop=(ec == ECH - 1))
                    nc.scalar.activation(
                        st_sb[:, j, q * NQ:(q + 1) * NQ], st_ps,
                        mybir.ActivationFunctionType.Exp, scale=SCALE)
                # running softmax denominators on the idle Vector engine
                nc.vector.tensor_tensor(
                    out=l_acc, in0=l_acc, in1=st_sb[:, j, :],
                    op=mybir.AluOpType.add)
                # v AllGathers fire from inside phase 1, where model DMA
                # demand is low — phase 0 then only carries the kt-AG HBM
                # bursts. The tiny gpsimd copy reading st_sb[., j, .] holds
                # the AG back in the gpsimd FIFO until this point.
                if j == 12 or j == 36:
                    h = 0 if j == 12 else 1
                    agd = res.tile([1, 2], bf16, name=f"agd_{h}")
                    nc.gpsimd.tensor_copy(out=agd, in_=st_sb[0:1, j, 0:2])
                    nc.gpsimd.collective_compute(
                        "AllGather", mybir.AluOpType.bypass,
                        replica_groups=groups,
                        ins=[v_in[h].opt()],
                        outs=[v_all[h].opt()])

            # --- phase 1b: partition-reduce l_acc, reciprocal ---------------
            l_ps = [ps.tile([1, NQ], f32, tag="mm", name=f"l_{q}")
                    for q in range(QCH)]
            for q in range(QCH):
                nc.tensor.matmul(l_ps[q], ones_sb,
                                 l_acc[:, q * NQ:(q + 1) * NQ],
                                 start=True, stop=True)
            l_row = res.tile([1, SL], f32, tag="lacc")
            for q in range(QCH):
                nc.vector.tensor_copy(out=l_row[:, q * NQ:(q + 1) * NQ],
                                      in_=l_ps[q])
            nc.scalar.dma_start(out=l_dram, in_=l_row)
            l_pp = res.tile([P, SCH], f32)
            nc.sync.dma_start(out=l_pp,
                              in_=l_dram[0].rearrange("(c p) -> p c", p=P))
            recip = res.tile([P, SCH], f32)
            nc.vector.reciprocal(recip, l_pp)
            _bv_ap = bv.ap()

            # --- phase 2: O = exp(ST)^T @ V, 8 passes of 4 PSUM banks -------
            for eq in range(EQN):
                bv_bcast_ap = bass.AP(tensor=_bv_ap.tensor,
                                      offset=_bv_ap.offset + eq * NQ,
                                      ap=[[0, P], [1, NQ]])
                bv_sb = stg.tile([P, NQ], bf16, tag="bv", name=f"bv_{eq}")
                nc.sync.dma_start(out=bv_sb, in_=bv_bcast_ap)
                for hf in range(2):
                    o_ps = [ps.tile([P, NQ], f32, tag="mm",
                                    name=f"o_{eq}_{hf}_{si}")
                            for si in range(4)]
                    for j in range(JN):
                        r, sloc = j // SCH, (j % SCH) * P
                        h, off = sloc // VSL, sloc % VSL
                        v_t = wstr.tile([P, NQ], bf16, tag="w",
                                        name=f"v_{eq}_{hf}_{j}")
                        nc.sync.dma_start(
                            out=v_t,
                            in_=v_all[h][r * VSL + off:r * VSL + off + P,
                                         eq * NQ:(eq + 1) * NQ])
                        for si in range(4):
                            s = hf * 4 + si
                            nc.tensor.matmul(
                                o_ps[si], st_sb[:, j, s * P:(s + 1) * P], v_t,
                                start=(j == 0), stop=(j == JN - 1))
                    for si in range(4):
                        s = hf * 4 + si
                        o_stg = stg.tile([P, NQ], f32, tag="ostg",
                                         name=f"ostg_{eq}_{hf}_{si}")
                        nc.vector.tensor_scalar_mul(o_stg, o_ps[si],
                                                    recip[:, s:s + 1])
                        nc.vector.tensor_tensor(
                            out=o_stg, in0=o_stg, in1=bv_sb,
                            op=mybir.AluOpType.add)
                        nc.scalar.dma_start(
                            out=out[s * P:(s + 1) * P, eq * NQ:(eq + 1) * NQ],
                            in_=o_stg)

    nc.compile()
    return nc


def kernel(x, Wq, bq, Wk, bk, Wv, bv):
    from concourse.bass_utils import run_bass_kernel_spmd

    xt = np.ascontiguousarray(x.astype(_BF16).T)          # [E, S] bf16

    def _pre(w):  # [e_in, e_out] -> [eo, p, c, n] so each eo-slice is contiguous
        return np.ascontiguousarray(
            w.astype(_BF16).reshape(ECH, P, ECH, P).transpose(2, 1, 0, 3))

    wqb = _pre(Wq)
    wkb = _pre(Wk)
    wvb = np.ascontiguousarray(Wv.astype(_BF16))
    bqf = np.ascontiguousarray(bq.astype(np.float32))
    bkf = np.ascontiguousarray(bk.astype(np.float32))
    bvf = np.ascontiguousarray(bv.astype(_BF16))

    in_maps = []
    for r in range(N_CORES):
        in_maps.append({
            "xt": np.ascontiguousarray(xt[:, r * SL:(r + 1) * SL]),
            "wq": wqb, "wk": wkb, "wv": wvb,
            "bq": bqf, "bk": bkf, "bv": bvf,
        })

    nc = _build()
    res = run_bass_kernel_spmd(nc, in_maps, core_ids=list(range(N_CORES)))
    global LAST_RESULT
    LAST_RESULT = res
    return np.concatenate([res.results[r]["out"] for r in range(N_CORES)],
                          axis=0).astype(np.float32)


LAST_RESULT = None


# revision 18
# speedup vs baseline: 1.0050x; 1.0050x over previous
"""Single-head self-attention (CrossVit block) on 8 Trainium2 NeuronCores.

Computation (fp32 reference):
    q = x @ Wq + bq ; k = x @ Wk + bk ; v = x @ Wv + bv        [S, E]
    scores = (q @ k^T) / sqrt(E)                               [S, S]
    out = softmax(scores, axis=-1) @ v                         [S, E]
with S = 8192, E = 2048.

Strategy (sequence-parallel over query rows, 1024 per core):
  Host: transpose x -> xT [E, S] (bf16) and hand each core its column
  slice xT_i [E, 1024] plus full Wq/Wk/Wv (bf16) and biases (fp32).
  Device, per core i:
    phase 0: KT_i = Wk^T xT_i (+bk) -> DRAM (4 separate slice tiles),
             each AllGather'd pipelined against the projection itself;
             V_i = xT_i Wv -> DRAM (2 separate slice tiles), AllGather'd;
             QT_i = Wq^T xT_i (+bq) stays in SBUF.
    phase 1: ST[j] = KT_j^T @ QT  (scores transposed: [sk, sq]) -> exp -> SBUF
             l_acc[p, sq] += exp(ST[j])   (running sums on the Vector engine)
    phase 1b: l[sq] = ones^T @ l_acc  (2 fp32 matmuls), transpose via DRAM,
             reciprocal.
    phase 2: O[sq, e] = sum_j exp(ST[j])^T @ V_j, 8 passes of 4 PSUM banks
             (eq x s-half) so epilogues overlap the next pass,
             epilogue: O * (1/l) + bv -> out
  Host: concatenate the 8 row blocks.
"""

import math

import numpy as np
import ml_dtypes

S = 8192
E = 2048
N_CORES = 8
SL = S // N_CORES      # 1024 query rows per core
P = 128                # partitions
ECH = E // P           # 16 contraction chunks
JN = S // P            # 64 global key chunks
NQ = 512               # moving free-dim (ISA max per matmul)
QCH = SL // NQ         # 2 query chunks of 512
SCH = SL // P          # 8 query chunks of 128
EQN = E // NQ          # 4 output-column chunks of 512
KAG = 4                # AllGather splits for KT (rows of 512)
VAG = 2                # AllGather splits for V (rows of 512)
EHALF = ECH // 2       # 8 contraction chunks per weight half-tile

_BF16 = ml_dtypes.bfloat16


def _build():
    import concourse.bacc as bacc
    import concourse.bass as bass
    import concourse.tile as tile
    import concourse.mybir as mybir

    bf16 = mybir.dt.bfloat16
    f32 = mybir.dt.float32
    SCALE = 1.0 / math.sqrt(float(E))

    nc = bacc.Bacc("TRN2", target_bir_lowering=False, debug=False,
                   num_devices=N_CORES)

    xt = nc.declare_dram_parameter("xt", [E, SL], bf16, isOutput=False)
    wq = nc.declare_dram_parameter("wq", [ECH, P, ECH, P], bf16, isOutput=False)
    wk = nc.declare_dram_parameter("wk", [ECH, P, ECH, P], bf16, isOutput=False)
    wv = nc.declare_dram_parameter("wv", [E, E], bf16, isOutput=False)
    bq = nc.declare_dram_parameter("bq", [E], f32, isOutput=False)
    bk = nc.declare_dram_parameter("bk", [E], f32, isOutput=False)
    bv = nc.declare_dram_parameter("bv", [E], bf16, isOutput=False)
    out = nc.declare_dram_parameter("out", [SL, E], f32, isOutput=True)

    groups = [list(range(N_CORES))]
    KSL = E // KAG        # 512 rows per KT AG slice
    VSL = SL // VAG       # 512 rows per V AG slice

    with tile.TileContext(nc) as tc:
        with (
            tc.tile_pool(name="dram", bufs=1, space="DRAM") as dram,
            tc.tile_pool(name="big", bufs=1) as big,
            tc.tile_pool(name="res", bufs=1) as res,
            tc.tile_pool(name="wstr", bufs=6) as wstr,
            tc.tile_pool(name="vstr", bufs=12) as vstr,
            tc.tile_pool(name="stg", bufs=2) as stg,
            tc.tile_pool(name="ps", bufs=8, space="PSUM") as ps,
        ):
            # separate staging tiles per AllGather slice: no write-after-read
            # hazard between projection slice i+1 and the AG of slice i.
            kt_in = [dram.tile([KSL, SL], bf16, name=f"kt_in_{i}")
                     for i in range(KAG)]
            v_in = [dram.tile([VSL, E], bf16, name=f"v_in_{h}")
                    for h in range(VAG)]
            kt_all = [dram.tile([N_CORES * KSL, SL], bf16, addr_space="Shared",
                                name=f"kt_all_{i}") for i in range(KAG)]
            v_all = [dram.tile([N_CORES * VSL, E], bf16, addr_space="Shared",
                               name=f"v_all_{i}") for i in range(VAG)]
            l_dram = dram.tile([1, SL], f32)

            # --- resident SBUF tensors -------------------------------------
            # xt_sb and st_sb share one 128KB/partition slot (disjoint
            # lifetimes: xt only in phase 0, st written in phase 1).
            xt_sb = big.tile([P, ECH, SL], bf16, tag="bigslot")
            qt_sb = res.tile([P, ECH, SL], bf16)
            bq_sb = res.tile([P, ECH], f32)
            bk_sb = res.tile([P, ECH], f32)
            ones_sb = res.tile([P, 1], f32)
            vsave = [res.tile([1, 2], bf16, name=f"vsave_{h}")
                     for h in range(VAG)]

            # startup: xt chunks 0-1, then the first weight tiles, then the
            # rest of xt — the first matmul can start after ~2 small DMAs.
            def _load_w(wtag, w_param, eo):
                w_h = []
                for hw in range(2):
                    t = wstr.tile([P, EHALF, P], bf16, tag="w",
                                  name=f"w_{wtag}_{eo}_{hw}")
                    nc.sync.dma_start(
                        out=t, in_=w_param[eo, :, hw * EHALF:(hw + 1) * EHALF])
                    w_h.append(t)
                return w_h

            for ec in range(ECH):
                nc.scalar.dma_start(out=xt_sb[:, ec],
                                    in_=xt[ec * P:(ec + 1) * P])
            w_pre = _load_w("wk", wk, 0)
            nc.scalar.dma_start(out=bq_sb, in_=bq.rearrange("(c p) -> p c", p=P))
            nc.scalar.dma_start(out=bk_sb, in_=bk.rearrange("(c p) -> p c", p=P))
            nc.vector.memset(ones_sb, 1.0)

            # --- phase 0a: KT_i = Wk^T @ xT_i + bk -> kt_in, AG in 4 slices -
            def qk_proj(w_param, b_sb, dst_sbuf, dst_dram, wtag, eo_lo, eo_hi,
                        w_pre=None):
                for eo in range(eo_lo, eo_hi):
                    # two half-eo weight tiles for finer prefetch granularity
                    if w_pre is not None and eo == eo_lo:
                        w_h = w_pre
                    else:
                        w_h = _load_w(wtag, w_param, eo)
                    for q in range(QCH):
                        acc = ps.tile([P, NQ], f32, tag="mm",
                                      name=f"acc_{wtag}_{eo}_{q}")
                        for ec in range(ECH):
                            nc.tensor.matmul(
                                acc, w_h[ec // EHALF][:, ec % EHALF],
                                xt_sb[:, ec, q * NQ:(q + 1) * NQ],
                                start=(ec == 0), stop=(ec == ECH - 1))
                        if dst_sbuf is not None:
                            nc.scalar.activation(
                                dst_sbuf[:, eo, q * NQ:(q + 1) * NQ], acc,
                                mybir.ActivationFunctionType.Identity,
                                bias=b_sb[:, eo:eo + 1], scale=1.0)
                        else:
                            kstg = stg.tile([P, NQ], bf16, tag="stg",
                                            name=f"kstg_{eo}_{q}")
                            nc.scalar.activation(
                                kstg, acc,
                                mybir.ActivationFunctionType.Identity,
                                bias=b_sb[:, eo:eo + 1], scale=1.0)
                            lo = (eo * P) % KSL
                            nc.scalar.dma_start(
                                out=dst_dram[lo:lo + P,
                                             q * NQ:(q + 1) * NQ],
                                in_=kstg)

            eo_per_slice = KSL // P  # 4
            for i in range(KAG):
                qk_proj(wk, bk_sb, None, kt_in[i], "wk",
                        i * eo_per_slice, (i + 1) * eo_per_slice,
                        w_pre=w_pre if i == 0 else None)
                nc.gpsimd.collective_compute(
                    "AllGather", mybir.AluOpType.bypass, replica_groups=groups,
                    ins=[kt_in[i].opt()],
                    outs=[kt_all[i].opt()])

            # --- phase 0b: V_i = xT_i^T @ Wv -> v_in -------------------------
            # bv is folded into the epilogue (attn rows sum to 1).
            # eq-major with all 8 s-chunks per pass: wv is streamed once
            # (8 MB, not 16) at half the rate, so a deeper wv runway rides
            # out the kt-AllGather HBM bursts. The v AllGathers fire later,
            # from inside phase 1 (see below).
            for eq in range(EQN):           # e-quarters of 512
                accs = [ps.tile([P, NQ], f32, tag="mm",
                                name=f"vacc_{eq}_{s}")
                        for s in range(SCH)]
                for ec in range(ECH):
                    wv_t = vstr.tile([P, NQ], bf16, tag="wv",
                                     name=f"wv_{eq}_{ec}")
                    nc.sync.dma_start(
                        out=wv_t,
                        in_=wv[ec * P:(ec + 1) * P,
                               eq * NQ:(eq + 1) * NQ])
                    for s in range(SCH):
                        nc.tensor.matmul(
                            accs[s], xt_sb[:, ec, s * P:(s + 1) * P],
                            wv_t, start=(ec == 0), stop=(ec == ECH - 1))
                for s in range(SCH):
                    vstg = stg.tile([P, NQ], bf16, tag="stg",
                                    name=f"vstg_{eq}_{s}")
                    nc.vector.tensor_copy(out=vstg, in_=accs[s])
                    h, lo = (s * P) // VSL, (s * P) % VSL
                    if eq == 0 and lo == 0:
                        # keep a copy of v_in[h][0, 0:2] to re-write later as
                        # the v-AllGather's delay dependency
                        nc.vector.tensor_copy(out=vsave[h],
                                              in_=vstg[0:1, 0:2])
                    nc.scalar.dma_start(
                        out=v_in[h][lo:lo + P, eq * NQ:(eq + 1) * NQ],
                        in_=vstg)

            # --- phase 0c: QT_i = Wq^T @ xT_i + bq -> qt_sb (SBUF-resident) -
            qk_proj(wq, bq_sb, qt_sb, None, "wq", 0, ECH)

            # --- phase 1: ST[j] = KT_j^T @ QT, exp; l_acc on Vector engine --
            st_sb = big.tile([P, JN, SL], bf16, tag="bigslot")
            l_acc = res.tile([P, SL], f32, tag="lacc")
            nc.vector.memset(l_acc, 0.0)
            ec_per_slice = ECH // KAG  # 4 e-chunks per AG slice
            for j in range(JN):
                r, c = j // SCH, j % SCH
                # shares the "w" tag: its buffer slot only frees late in
                # phase 0, which pins this DMA's static queue position past
                # the AllGathers (a hoisted AG-dependent DMA head-of-line
                # blocks the whole sync queue).
                kt_t = wstr.tile([P, ECH, P], bf16, tag="w", name=f"kt_{j}")
                for i in range(KAG):
                    nc.sync.dma_start(
                        out=kt_t[:, i * ec_per_slice:(i + 1) * ec_per_slice, :],
                        in_=kt_all[i][r * KSL:(r + 1) * KSL,
                                      c * P:(c + 1) * P].rearrange(
                                          "(ec p) s -> p ec s", p=P))
                for q in range(QCH):
                    st_ps = ps.tile([P, NQ], f32, tag="mm", name=f"st_{j}_{q}")
                    for ec in range(ECH):
                        nc.tensor.matmul(
                            st_ps, kt_t[:, ec],
                            qt_sb[:, ec, q * NQ:(q + 1) * NQ],
                            start=(ec == 0), stop=(ec == ECH - 1))
                    nc.scalar.activation(
                        st_sb[:, j, q * NQ:(q + 1) * NQ], st_ps,
                        mybir.ActivationFunctionType.Exp, scale=SCALE)
                # running softmax denominators on the idle Vector engine
                nc.vector.tensor_tensor(
                    out=l_acc, in0=l_acc, in1=st_sb[:, j, :],
                    op=mybir.AluOpType.add)
                # v AllGathers fire from inside phase 1, where model DMA
                # demand is low — phase 0 then only carries the kt-AG HBM
                # bursts. The AG's input is made to genuinely depend on
                # st_sb[., j, .]: re-write 2 bytes of v_in with identical
                # values computed as 0*st + vsave (exact).
                if j == 12 or j == 36:
                    h = 0 if j == 12 else 1
                    vz = res.tile([1, 2], bf16, name=f"vz_{h}")
                    nc.vector.tensor_scalar_mul(vz, st_sb[0:1, j, 0:2], 0.0)
                    nc.vector.tensor_tensor(out=vz, in0=vz, in1=vsave[h],
                                            op=mybir.AluOpType.add)
                    nc.scalar.dma_start(out=v_in[h][0:1, 0:2], in_=vz)
                    nc.gpsimd.collective_compute(
                        "AllGather", mybir.AluOpType.bypass,
                        replica_groups=groups,
                        ins=[v_in[h].opt()],
                        outs=[v_all[h].opt()])

            # --- phase 1b: partition-reduce l_acc, reciprocal ---------------
            l_ps = [ps.tile([1, NQ], f32, tag="mm", name=f"l_{q}")
                    for q in range(QCH)]
            for q in range(QCH):
                nc.tensor.matmul(l_ps[q], ones_sb,
                                 l_acc[:, q * NQ:(q + 1) * NQ],
                                 start=True, stop=True)
            l_row = res.tile([1, SL], f32, tag="lacc")
            for q in range(QCH):
                nc.vector.tensor_copy(out=l_row[:, q * NQ:(q + 1) * NQ],
                                      in_=l_ps[q])
            nc.scalar.dma_start(out=l_dram, in_=l_row)
            l_pp = res.tile([P, SCH], f32)
            nc.sync.dma_start(out=l_pp,
                              in_=l_dram[0].rearrange("(c p) -> p c", p=P))
            recip = res.tile([P, SCH], f32)
            nc.vector.reciprocal(recip, l_pp)
            _bv_ap = bv.ap()

            # --- phase 2: O = exp(ST)^T @ V, 8 passes of 4 PSUM banks -------
            for eq in range(EQN):
                bv_bcast_ap = bass.AP(tensor=_bv_ap.tensor,
                                      offset=_bv_ap.offset + eq * NQ,
                                      ap=[[0, P], [1, NQ]])
                bv_sb = stg.tile([P, NQ], bf16, tag="bv", name=f"bv_{eq}")
                nc.sync.dma_start(out=bv_sb, in_=bv_bcast_ap)
                for hf in range(2):
                    o_ps = [ps.tile([P, NQ], f32, tag="mm",
                                    name=f"o_{eq}_{hf}_{si}")
                            for si in range(4)]
                    for j in range(JN):
                        r, sloc = j // SCH, (j % SCH) * P
                        h, off = sloc // VSL, sloc % VSL
                        v_t = wstr.tile([P, NQ], bf16, tag="w",
                                        name=f"v_{eq}_{hf}_{j}")
                        nc.sync.dma_start(
                            out=v_t,
                            in_=v_all[h][r * VSL + off:r * VSL + off + P,
                                         eq * NQ:(eq + 1) * NQ])
                        for si in range(4):
                            s = hf * 4 + si
                            nc.tensor.matmul(
                                o_ps[si], st_sb[:, j, s * P:(s + 1) * P], v_t,
                                start=(j == 0), stop=(j == JN - 1))
                    for si in range(4):
                        s = hf * 4 + si
                        o_stg = stg.tile([P, NQ], f32, tag="stg",
                                         name=f"ostg_{eq}_{hf}_{si}")
                        nc.vector.tensor_scalar_mul(o_stg, o_ps[si],
                                                    recip[:, s:s + 1])
                        nc.vector.tensor_tensor(
                            out=o_stg, in0=o_stg, in1=bv_sb,
                            op=mybir.AluOpType.add)
                        nc.scalar.dma_start(
                            out=out[s * P:(s + 1) * P, eq * NQ:(eq + 1) * NQ],
                            in_=o_stg)

    nc.compile()
    return nc


def kernel(x, Wq, bq, Wk, bk, Wv, bv):
    from concourse.bass_utils import run_bass_kernel_spmd

    xt = np.ascontiguousarray(x.astype(_BF16).T)          # [E, S] bf16

    def _pre(w):  # [e_in, e_out] -> [eo, p, c, n] so each eo-slice is contiguous
        return np.ascontiguousarray(
            w.astype(_BF16).reshape(ECH, P, ECH, P).transpose(2, 1, 0, 3))

    wqb = _pre(Wq)
    wkb = _pre(Wk)
    wvb = np.ascontiguousarray(Wv.astype(_BF16))
    bqf = np.ascontiguousarray(bq.astype(np.float32))
    bkf = np.ascontiguousarray(bk.astype(np.float32))
    bvf = np.ascontiguousarray(bv.astype(_BF16))

    in_maps = []
    for r in range(N_CORES):
        in_maps.append({
            "xt": np.ascontiguousarray(xt[:, r * SL:(r + 1) * SL]),
            "wq": wqb, "wk": wkb, "wv": wvb,
            "bq": bqf, "bk": bkf, "bv": bvf,
        })

    nc = _build()
    res = run_bass_kernel_spmd(nc, in_maps, core_ids=list(range(N_CORES)))
    global LAST_RESULT
    LAST_RESULT = res
    return np.concatenate([res.results[r]["out"] for r in range(N_CORES)],
                          axis=0).astype(np.float32)


LAST_RESULT = None


# revision 19
# speedup vs baseline: 1.0111x; 1.0061x over previous
"""Single-head self-attention (CrossVit block) on 8 Trainium2 NeuronCores.

Computation (fp32 reference):
    q = x @ Wq + bq ; k = x @ Wk + bk ; v = x @ Wv + bv        [S, E]
    scores = (q @ k^T) / sqrt(E)                               [S, S]
    out = softmax(scores, axis=-1) @ v                         [S, E]
with S = 8192, E = 2048.

Strategy (sequence-parallel over query rows, 1024 per core):
  Host: transpose x -> xT [E, S] (bf16) and hand each core its column
  slice xT_i [E, 1024] plus full Wq/Wk/Wv (bf16) and biases (fp32).
  Device, per core i:
    phase 0: KT_i = Wk^T xT_i (+bk) -> DRAM (4 separate slice tiles),
             each AllGather'd pipelined against the projection itself;
             V_i = xT_i Wv -> DRAM (2 separate slice tiles), AllGather'd;
             QT_i = Wq^T xT_i (+bq) stays in SBUF.
    phase 1: ST[j] = KT_j^T @ QT  (scores transposed: [sk, sq]) -> exp -> SBUF
             l_acc[p, sq] += exp(ST[j])   (running sums on the Vector engine)
    phase 1b: l[sq] = ones^T @ l_acc  (2 fp32 matmuls), transpose via DRAM,
             reciprocal.
    phase 2: O[sq, e] = sum_j exp(ST[j])^T @ V_j, 8 passes of 4 PSUM banks
             (eq x s-half) so epilogues overlap the next pass,
             epilogue: O * (1/l) + bv -> out
  Host: concatenate the 8 row blocks.
"""

import math

import numpy as np
import ml_dtypes

S = 8192
E = 2048
N_CORES = 8
SL = S // N_CORES      # 1024 query rows per core
P = 128                # partitions
ECH = E // P           # 16 contraction chunks
JN = S // P            # 64 global key chunks
NQ = 512               # moving free-dim (ISA max per matmul)
QCH = SL // NQ         # 2 query chunks of 512
SCH = SL // P          # 8 query chunks of 128
EQN = E // NQ          # 4 output-column chunks of 512
KAG = 4                # AllGather splits for KT (rows of 512)
VAG = 2                # AllGather splits for V (rows of 512)
EHALF = ECH // 2       # 8 contraction chunks per weight half-tile

_BF16 = ml_dtypes.bfloat16


def _build():
    import concourse.bacc as bacc
    import concourse.bass as bass
    import concourse.tile as tile
    import concourse.mybir as mybir

    bf16 = mybir.dt.bfloat16
    f32 = mybir.dt.float32
    SCALE = 1.0 / math.sqrt(float(E))

    nc = bacc.Bacc("TRN2", target_bir_lowering=False, debug=False,
                   num_devices=N_CORES)

    xt = nc.declare_dram_parameter("xt", [E, SL], bf16, isOutput=False)
    wq = nc.declare_dram_parameter("wq", [ECH, P, ECH, P], bf16, isOutput=False)
    wk = nc.declare_dram_parameter("wk", [ECH, P, ECH, P], bf16, isOutput=False)
    wv = nc.declare_dram_parameter("wv", [E, E], bf16, isOutput=False)
    bq = nc.declare_dram_parameter("bq", [E], f32, isOutput=False)
    bk = nc.declare_dram_parameter("bk", [E], f32, isOutput=False)
    bv = nc.declare_dram_parameter("bv", [E], bf16, isOutput=False)
    out = nc.declare_dram_parameter("out", [SL, E], f32, isOutput=True)

    groups = [list(range(N_CORES))]
    KSL = E // KAG        # 512 rows per KT AG slice
    VSL = SL // VAG       # 512 rows per V AG slice

    with tile.TileContext(nc) as tc:
        with (
            tc.tile_pool(name="dram", bufs=1, space="DRAM") as dram,
            tc.tile_pool(name="big", bufs=1) as big,
            tc.tile_pool(name="res", bufs=1) as res,
            tc.tile_pool(name="wstr", bufs=6) as wstr,
            tc.tile_pool(name="vstr", bufs=12) as vstr,
            tc.tile_pool(name="stg", bufs=2) as stg,
            tc.tile_pool(name="ps", bufs=8, space="PSUM") as ps,
        ):
            # separate staging tiles per AllGather slice: no write-after-read
            # hazard between projection slice i+1 and the AG of slice i.
            kt_in = [dram.tile([KSL, SL], bf16, name=f"kt_in_{i}")
                     for i in range(KAG)]
            v_in = [dram.tile([VSL, E], bf16, name=f"v_in_{h}")
                    for h in range(VAG)]
            kt_all = [dram.tile([N_CORES * KSL, SL], bf16, addr_space="Shared",
                                name=f"kt_all_{i}") for i in range(KAG)]
            v_all = [dram.tile([N_CORES * VSL, E], bf16, addr_space="Shared",
                               name=f"v_all_{i}") for i in range(VAG)]
            l_dram = dram.tile([1, SL], f32)

            # --- resident SBUF tensors -------------------------------------
            # xt_sb and st_sb share one 128KB/partition slot (disjoint
            # lifetimes: xt only in phase 0, st written in phase 1).
            xt_sb = big.tile([P, ECH, SL], bf16, tag="bigslot")
            qt_sb = res.tile([P, ECH, SL], bf16)
            bq_sb = res.tile([P, ECH], f32)
            bk_sb = res.tile([P, ECH], f32)
            ones_sb = res.tile([P, 1], f32)
            vsave = [res.tile([1, 2], bf16, name=f"vsave_{h}")
                     for h in range(VAG)]

            # startup: xt chunks 0-1, then the first weight tiles, then the
            # rest of xt — the first matmul can start after ~2 small DMAs.
            def _load_w(wtag, w_param, eo):
                w_h = []
                for hw in range(2):
                    t = wstr.tile([P, EHALF, P], bf16, tag="w",
                                  name=f"w_{wtag}_{eo}_{hw}")
                    nc.sync.dma_start(
                        out=t, in_=w_param[eo, :, hw * EHALF:(hw + 1) * EHALF])
                    w_h.append(t)
                return w_h

            for ec in range(ECH):
                eng = nc.scalar if ec % 2 == 0 else nc.sync
                eng.dma_start(out=xt_sb[:, ec],
                              in_=xt[ec * P:(ec + 1) * P])
            w_pre = _load_w("wk", wk, 0)
            nc.scalar.dma_start(out=bq_sb, in_=bq.rearrange("(c p) -> p c", p=P))
            nc.scalar.dma_start(out=bk_sb, in_=bk.rearrange("(c p) -> p c", p=P))
            nc.vector.memset(ones_sb, 1.0)

            # --- phase 0a: KT_i = Wk^T @ xT_i + bk -> kt_in, AG in 4 slices -
            def qk_proj(w_param, b_sb, dst_sbuf, dst_dram, wtag, eo_lo, eo_hi,
                        w_pre=None):
                for eo in range(eo_lo, eo_hi):
                    # two half-eo weight tiles for finer prefetch granularity
                    if w_pre is not None and eo == eo_lo:
                        w_h = w_pre
                    else:
                        w_h = _load_w(wtag, w_param, eo)
                    for q in range(QCH):
                        acc = ps.tile([P, NQ], f32, tag="mm",
                                      name=f"acc_{wtag}_{eo}_{q}")
                        for ec in range(ECH):
                            nc.tensor.matmul(
                                acc, w_h[ec // EHALF][:, ec % EHALF],
                                xt_sb[:, ec, q * NQ:(q + 1) * NQ],
                                start=(ec == 0), stop=(ec == ECH - 1))
                        if dst_sbuf is not None:
                            nc.scalar.activation(
                                dst_sbuf[:, eo, q * NQ:(q + 1) * NQ], acc,
                                mybir.ActivationFunctionType.Identity,
                                bias=b_sb[:, eo:eo + 1], scale=1.0)
                        else:
                            kstg = stg.tile([P, NQ], bf16, tag="stg",
                                            name=f"kstg_{eo}_{q}")
                            nc.scalar.activation(
                                kstg, acc,
                                mybir.ActivationFunctionType.Identity,
                                bias=b_sb[:, eo:eo + 1], scale=1.0)
                            lo = (eo * P) % KSL
                            nc.scalar.dma_start(
                                out=dst_dram[lo:lo + P,
                                             q * NQ:(q + 1) * NQ],
                                in_=kstg)

            eo_per_slice = KSL // P  # 4
            for i in range(KAG):
                qk_proj(wk, bk_sb, None, kt_in[i], "wk",
                        i * eo_per_slice, (i + 1) * eo_per_slice,
                        w_pre=w_pre if i == 0 else None)
                nc.gpsimd.collective_compute(
                    "AllGather", mybir.AluOpType.bypass, replica_groups=groups,
                    ins=[kt_in[i].opt()],
                    outs=[kt_all[i].opt()])

            # --- phase 0b: V_i = xT_i^T @ Wv -> v_in -------------------------
            # bv is folded into the epilogue (attn rows sum to 1).
            # eq-major with all 8 s-chunks per pass: wv is streamed once
            # (8 MB, not 16) at half the rate, so a deeper wv runway rides
            # out the kt-AllGather HBM bursts. The v AllGathers fire later,
            # from inside phase 1 (see below).
            for eq in range(EQN):           # e-quarters of 512
                accs = [ps.tile([P, NQ], f32, tag="mm",
                                name=f"vacc_{eq}_{s}")
                        for s in range(SCH)]
                for ec in range(ECH):
                    wv_t = vstr.tile([P, NQ], bf16, tag="wv",
                                     name=f"wv_{eq}_{ec}")
                    eng = nc.sync if ec % 2 == 0 else nc.scalar
                    eng.dma_start(
                        out=wv_t,
                        in_=wv[ec * P:(ec + 1) * P,
                               eq * NQ:(eq + 1) * NQ])
                    for s in range(SCH):
                        nc.tensor.matmul(
                            accs[s], xt_sb[:, ec, s * P:(s + 1) * P],
                            wv_t, start=(ec == 0), stop=(ec == ECH - 1))
                for s in range(SCH):
                    vstg = stg.tile([P, NQ], bf16, tag="stg",
                                    name=f"vstg_{eq}_{s}")
                    nc.vector.tensor_copy(out=vstg, in_=accs[s])
                    h, lo = (s * P) // VSL, (s * P) % VSL
                    if eq == 0 and lo == 0:
                        # keep a copy of v_in[h][0, 0:2] to re-write later as
                        # the v-AllGather's delay dependency
                        nc.vector.tensor_copy(out=vsave[h],
                                              in_=vstg[0:1, 0:2])
                    nc.scalar.dma_start(
                        out=v_in[h][lo:lo + P, eq * NQ:(eq + 1) * NQ],
                        in_=vstg)

            # --- phase 0c: QT_i = Wq^T @ xT_i + bq -> qt_sb (SBUF-resident) -
            qk_proj(wq, bq_sb, qt_sb, None, "wq", 0, ECH)

            # --- phase 1: ST[j] = KT_j^T @ QT, exp; l_acc on Vector engine --
            st_sb = big.tile([P, JN, SL], bf16, tag="bigslot")
            l_acc = res.tile([P, SL], f32, tag="lacc")
            nc.vector.memset(l_acc, 0.0)
            ec_per_slice = ECH // KAG  # 4 e-chunks per AG slice
            for j in range(JN):
                r, c = j // SCH, j % SCH
                # shares the "w" tag: its buffer slot only frees late in
                # phase 0, which pins this DMA's static queue position past
                # the AllGathers (a hoisted AG-dependent DMA head-of-line
                # blocks the whole sync queue).
                kt_t = wstr.tile([P, ECH, P], bf16, tag="w", name=f"kt_{j}")
                for i in range(KAG):
                    nc.sync.dma_start(
                        out=kt_t[:, i * ec_per_slice:(i + 1) * ec_per_slice, :],
                        in_=kt_all[i][r * KSL:(r + 1) * KSL,
                                      c * P:(c + 1) * P].rearrange(
                                          "(ec p) s -> p ec s", p=P))
                for q in range(QCH):
                    st_ps = ps.tile([P, NQ], f32, tag="mm", name=f"st_{j}_{q}")
                    for ec in range(ECH):
                        nc.tensor.matmul(
                            st_ps, kt_t[:, ec],
                            qt_sb[:, ec, q * NQ:(q + 1) * NQ],
                            start=(ec == 0), stop=(ec == ECH - 1))
                    nc.scalar.activation(
                        st_sb[:, j, q * NQ:(q + 1) * NQ], st_ps,
                        mybir.ActivationFunctionType.Exp, scale=SCALE)
                # running softmax denominators on the idle Vector engine
                nc.vector.tensor_tensor(
                    out=l_acc, in0=l_acc, in1=st_sb[:, j, :],
                    op=mybir.AluOpType.add)
                # v AllGathers fire from inside phase 1, where model DMA
                # demand is low — phase 0 then only carries the kt-AG HBM
                # bursts. The AG's input is made to genuinely depend on
                # st_sb[., j, .]: re-write 2 bytes of v_in with identical
                # values computed as 0*st + vsave (exact).
                if j == 12 or j == 36:
                    h = 0 if j == 12 else 1
                    vz = res.tile([1, 2], bf16, name=f"vz_{h}")
                    nc.vector.tensor_scalar_mul(vz, st_sb[0:1, j, 0:2], 0.0)
                    nc.vector.tensor_tensor(out=vz, in0=vz, in1=vsave[h],
                                            op=mybir.AluOpType.add)
                    nc.scalar.dma_start(out=v_in[h][0:1, 0:2], in_=vz)
                    nc.gpsimd.collective_compute(
                        "AllGather", mybir.AluOpType.bypass,
                        replica_groups=groups,
                        ins=[v_in[h].opt()],
                        outs=[v_all[h].opt()])

            # --- phase 1b: partition-reduce l_acc, reciprocal ---------------
            l_ps = [ps.tile([1, NQ], f32, tag="mm", name=f"l_{q}")
                    for q in range(QCH)]
            for q in range(QCH):
                nc.tensor.matmul(l_ps[q], ones_sb,
                                 l_acc[:, q * NQ:(q + 1) * NQ],
                                 start=True, stop=True)
            l_row = res.tile([1, SL], f32, tag="lacc")
            for q in range(QCH):
                nc.vector.tensor_copy(out=l_row[:, q * NQ:(q + 1) * NQ],
                                      in_=l_ps[q])
            nc.scalar.dma_start(out=l_dram, in_=l_row)
            l_pp = res.tile([P, SCH], f32)
            nc.sync.dma_start(out=l_pp,
                              in_=l_dram[0].rearrange("(c p) -> p c", p=P))
            recip = res.tile([P, SCH], f32)
            nc.vector.reciprocal(recip, l_pp)
            _bv_ap = bv.ap()

            # --- phase 2: O = exp(ST)^T @ V, 8 passes of 4 PSUM banks -------
            for eq in range(EQN):
                bv_bcast_ap = bass.AP(tensor=_bv_ap.tensor,
                                      offset=_bv_ap.offset + eq * NQ,
                                      ap=[[0, P], [1, NQ]])
                bv_sb = stg.tile([P, NQ], bf16, tag="bv", name=f"bv_{eq}")
                nc.sync.dma_start(out=bv_sb, in_=bv_bcast_ap)
                for hf in range(2):
                    o_ps = [ps.tile([P, NQ], f32, tag="mm",
                                    name=f"o_{eq}_{hf}_{si}")
                            for si in range(4)]
                    for j in range(JN):
                        r, sloc = j // SCH, (j % SCH) * P
                        h, off = sloc // VSL, sloc % VSL
                        v_t = wstr.tile([P, NQ], bf16, tag="w",
                                        name=f"v_{eq}_{hf}_{j}")
                        nc.sync.dma_start(
                            out=v_t,
                            in_=v_all[h][r * VSL + off:r * VSL + off + P,
                                         eq * NQ:(eq + 1) * NQ])
                        for si in range(4):
                            s = hf * 4 + si
                            nc.tensor.matmul(
                                o_ps[si], st_sb[:, j, s * P:(s + 1) * P], v_t,
                                start=(j == 0), stop=(j == JN - 1))
                    for si in range(4):
                        s = hf * 4 + si
                        o_stg = stg.tile([P, NQ], f32, tag="stg",
                                         name=f"ostg_{eq}_{hf}_{si}")
                        nc.vector.tensor_scalar_mul(o_stg, o_ps[si],
                                                    recip[:, s:s + 1])
                        nc.vector.tensor_tensor(
                            out=o_stg, in0=o_stg, in1=bv_sb,
                            op=mybir.AluOpType.add)
                        nc.scalar.dma_start(
                            out=out[s * P:(s + 1) * P, eq * NQ:(eq + 1) * NQ],
                            in_=o_stg)

    nc.compile()
    return nc


def kernel(x, Wq, bq, Wk, bk, Wv, bv):
    from concourse.bass_utils import run_bass_kernel_spmd

    xt = np.ascontiguousarray(x.astype(_BF16).T)          # [E, S] bf16

    def _pre(w):  # [e_in, e_out] -> [eo, p, c, n] so each eo-slice is contiguous
        return np.ascontiguousarray(
            w.astype(_BF16).reshape(ECH, P, ECH, P).transpose(2, 1, 0, 3))

    wqb = _pre(Wq)
    wkb = _pre(Wk)
    wvb = np.ascontiguousarray(Wv.astype(_BF16))
    bqf = np.ascontiguousarray(bq.astype(np.float32))
    bkf = np.ascontiguousarray(bk.astype(np.float32))
    bvf = np.ascontiguousarray(bv.astype(_BF16))

    in_maps = []
    for r in range(N_CORES):
        in_maps.append({
            "xt": np.ascontiguousarray(xt[:, r * SL:(r + 1) * SL]),
            "wq": wqb, "wk": wkb, "wv": wvb,
            "bq": bqf, "bk": bkf, "bv": bvf,
        })

    nc = _build()
    res = run_bass_kernel_spmd(nc, in_maps, core_ids=list(range(N_CORES)))
    global LAST_RESULT
    LAST_RESULT = res
    return np.concatenate([res.results[r]["out"] for r in range(N_CORES)],
                          axis=0).astype(np.float32)


LAST_RESULT = None


# revision 20
# speedup vs baseline: 1.0155x; 1.0043x over previous
"""Single-head self-attention (CrossVit block) on 8 Trainium2 NeuronCores.

Computation (fp32 reference):
    q = x @ Wq + bq ; k = x @ Wk + bk ; v = x @ Wv + bv        [S, E]
    scores = (q @ k^T) / sqrt(E)                               [S, S]
    out = softmax(scores, axis=-1) @ v                         [S, E]
with S = 8192, E = 2048.

Strategy (sequence-parallel over query rows, 1024 per core):
  Host: transpose x -> xT [E, S] (bf16) and hand each core its column
  slice xT_i [E, 1024] plus full Wq/Wk/Wv (bf16) and biases (fp32).
  Device, per core i:
    phase 0: KT_i = Wk^T xT_i (+bk) -> DRAM (4 separate slice tiles),
             each AllGather'd pipelined against the projection itself;
             V_i = xT_i Wv -> DRAM (2 separate slice tiles), AllGather'd;
             QT_i = Wq^T xT_i (+bq) stays in SBUF.
    phase 1: ST[j] = KT_j^T @ QT  (scores transposed: [sk, sq]) -> exp -> SBUF
             l_acc[p, sq] += exp(ST[j])   (running sums on the Vector engine)
    phase 1b: l[sq] = ones^T @ l_acc  (2 fp32 matmuls), transpose via DRAM,
             reciprocal.
    phase 2: O[sq, e] = sum_j exp(ST[j])^T @ V_j, 8 passes of 4 PSUM banks
             (eq x s-half) so epilogues overlap the next pass,
             epilogue: O * (1/l) + bv -> out
  Host: concatenate the 8 row blocks.
"""

import math

import numpy as np
import ml_dtypes

S = 8192
E = 2048
N_CORES = 8
SL = S // N_CORES      # 1024 query rows per core
P = 128                # partitions
ECH = E // P           # 16 contraction chunks
JN = S // P            # 64 global key chunks
NQ = 512               # moving free-dim (ISA max per matmul)
QCH = SL // NQ         # 2 query chunks of 512
SCH = SL // P          # 8 query chunks of 128
EQN = E // NQ          # 4 output-column chunks of 512
KAG = 4                # AllGather splits for KT (rows of 512)
VAG = 2                # AllGather splits for V (rows of 512)
EHALF = ECH // 2       # 8 contraction chunks per weight half-tile

_BF16 = ml_dtypes.bfloat16


def _build():
    import concourse.bacc as bacc
    import concourse.bass as bass
    import concourse.tile as tile
    import concourse.mybir as mybir

    bf16 = mybir.dt.bfloat16
    f32 = mybir.dt.float32
    SCALE = 1.0 / math.sqrt(float(E))

    nc = bacc.Bacc("TRN2", target_bir_lowering=False, debug=False,
                   num_devices=N_CORES)

    xt = nc.declare_dram_parameter("xt", [E, SL], bf16, isOutput=False)
    wq = nc.declare_dram_parameter("wq", [ECH, P, ECH, P], bf16, isOutput=False)
    wk = nc.declare_dram_parameter("wk", [ECH, P, ECH, P], bf16, isOutput=False)
    wv = nc.declare_dram_parameter("wv", [E, E], bf16, isOutput=False)
    bq = nc.declare_dram_parameter("bq", [E], f32, isOutput=False)
    bk = nc.declare_dram_parameter("bk", [E], f32, isOutput=False)
    bv = nc.declare_dram_parameter("bv", [E], bf16, isOutput=False)
    out = nc.declare_dram_parameter("out", [SL, E], f32, isOutput=True)

    groups = [list(range(N_CORES))]
    KSL = E // KAG        # 512 rows per KT AG slice
    VSL = SL // VAG       # 512 rows per V AG slice

    with tile.TileContext(nc) as tc:
        with (
            tc.tile_pool(name="dram", bufs=1, space="DRAM") as dram,
            tc.tile_pool(name="big", bufs=1) as big,
            tc.tile_pool(name="res", bufs=1) as res,
            tc.tile_pool(name="wstr", bufs=6) as wstr,
            tc.tile_pool(name="vstr", bufs=12) as vstr,
            tc.tile_pool(name="stg", bufs=2) as stg,
            tc.tile_pool(name="ps", bufs=8, space="PSUM") as ps,
        ):
            # separate staging tiles per AllGather slice: no write-after-read
            # hazard between projection slice i+1 and the AG of slice i.
            kt_in = [dram.tile([KSL, SL], bf16, name=f"kt_in_{i}")
                     for i in range(KAG)]
            v_in = [dram.tile([VSL, E], bf16, name=f"v_in_{h}")
                    for h in range(VAG)]
            kt_all = [dram.tile([N_CORES * KSL, SL], bf16, addr_space="Shared",
                                name=f"kt_all_{i}") for i in range(KAG)]
            v_all = [dram.tile([N_CORES * VSL, E], bf16, addr_space="Shared",
                               name=f"v_all_{i}") for i in range(VAG)]
            l_dram = dram.tile([1, SL], f32)

            # --- resident SBUF tensors -------------------------------------
            # xt_sb and st_sb share one 128KB/partition slot (disjoint
            # lifetimes: xt only in phase 0, st written in phase 1).
            xt_sb = big.tile([P, ECH, SL], bf16, tag="bigslot")
            qt_sb = res.tile([P, ECH, SL], bf16)
            bq_sb = res.tile([P, ECH], f32)
            bk_sb = res.tile([P, ECH], f32)
            ones_sb = res.tile([P, 1], f32)
            vsave = [res.tile([1, 2], bf16, name=f"vsave_{h}")
                     for h in range(VAG)]

            # startup: xt chunks 0-1, then the first weight tiles, then the
            # rest of xt — the first matmul can start after ~2 small DMAs.
            def _load_w(wtag, w_param, eo):
                w_h = []
                for hw in range(2):
                    t = wstr.tile([P, EHALF, P], bf16, tag="w",
                                  name=f"w_{wtag}_{eo}_{hw}")
                    nc.sync.dma_start(
                        out=t, in_=w_param[eo, :, hw * EHALF:(hw + 1) * EHALF])
                    w_h.append(t)
                return w_h

            for ec in range(ECH):
                eng = nc.scalar if ec % 2 == 0 else nc.sync
                eng.dma_start(out=xt_sb[:, ec],
                              in_=xt[ec * P:(ec + 1) * P])
            w_pre = _load_w("wk", wk, 0)
            nc.scalar.dma_start(out=bq_sb, in_=bq.rearrange("(c p) -> p c", p=P))
            nc.scalar.dma_start(out=bk_sb, in_=bk.rearrange("(c p) -> p c", p=P))
            nc.vector.memset(ones_sb, 1.0)

            # --- phase 0a: KT_i = Wk^T @ xT_i + bk -> kt_in, AG in 4 slices -
            def qk_proj(w_param, b_sb, dst_sbuf, dst_dram, wtag, eo_lo, eo_hi,
                        w_pre=None):
                for eo in range(eo_lo, eo_hi):
                    # two half-eo weight tiles for finer prefetch granularity
                    if w_pre is not None and eo == eo_lo:
                        w_h = w_pre
                    else:
                        w_h = _load_w(wtag, w_param, eo)
                    for q in range(QCH):
                        acc = ps.tile([P, NQ], f32, tag="mm",
                                      name=f"acc_{wtag}_{eo}_{q}")
                        for ec in range(ECH):
                            nc.tensor.matmul(
                                acc, w_h[ec // EHALF][:, ec % EHALF],
                                xt_sb[:, ec, q * NQ:(q + 1) * NQ],
                                start=(ec == 0), stop=(ec == ECH - 1))
                        if dst_sbuf is not None:
                            nc.scalar.activation(
                                dst_sbuf[:, eo, q * NQ:(q + 1) * NQ], acc,
                                mybir.ActivationFunctionType.Identity,
                                bias=b_sb[:, eo:eo + 1], scale=1.0)
                        else:
                            kstg = stg.tile([P, NQ], bf16, tag="stg",
                                            name=f"kstg_{eo}_{q}")
                            nc.scalar.activation(
                                kstg, acc,
                                mybir.ActivationFunctionType.Identity,
                                bias=b_sb[:, eo:eo + 1], scale=1.0)
                            lo = (eo * P) % KSL
                            nc.scalar.dma_start(
                                out=dst_dram[lo:lo + P,
                                             q * NQ:(q + 1) * NQ],
                                in_=kstg)

            eo_per_slice = KSL // P  # 4
            for i in range(KAG):
                qk_proj(wk, bk_sb, None, kt_in[i], "wk",
                        i * eo_per_slice, (i + 1) * eo_per_slice,
                        w_pre=w_pre if i == 0 else None)
                if i == KAG - 1:
                    continue  # kt AG 3 is delayed into late V-proj (below)
                nc.gpsimd.collective_compute(
                    "AllGather", mybir.AluOpType.bypass, replica_groups=groups,
                    ins=[kt_in[i].opt()],
                    outs=[kt_all[i].opt()])
            # readback of kt_in[3][0, 0:2] for the delayed-AG dependency
            ksb = res.tile([1, 2], bf16, name="ksb")
            nc.sync.dma_start(out=ksb, in_=kt_in[KAG - 1][0:1, 0:2])

            # --- phase 0b: V_i = xT_i^T @ Wv -> v_in -------------------------
            # bv is folded into the epilogue (attn rows sum to 1).
            # eq-major with all 8 s-chunks per pass: wv is streamed once
            # (8 MB, not 16) at half the rate, so a deeper wv runway rides
            # out the kt-AllGather HBM bursts. The v AllGathers fire later,
            # from inside phase 1 (see below).
            for eq in range(EQN):           # e-quarters of 512
                accs = [ps.tile([P, NQ], f32, tag="mm",
                                name=f"vacc_{eq}_{s}")
                        for s in range(SCH)]
                for ec in range(ECH):
                    wv_t = vstr.tile([P, NQ], bf16, tag="wv",
                                     name=f"wv_{eq}_{ec}")
                    eng = nc.sync if ec % 2 == 0 else nc.scalar
                    eng.dma_start(
                        out=wv_t,
                        in_=wv[ec * P:(ec + 1) * P,
                               eq * NQ:(eq + 1) * NQ])
                    for s in range(SCH):
                        nc.tensor.matmul(
                            accs[s], xt_sb[:, ec, s * P:(s + 1) * P],
                            wv_t, start=(ec == 0), stop=(ec == ECH - 1))
                for s in range(SCH):
                    vstg = stg.tile([P, NQ], bf16, tag="stg",
                                    name=f"vstg_{eq}_{s}")
                    nc.vector.tensor_copy(out=vstg, in_=accs[s])
                    h, lo = (s * P) // VSL, (s * P) % VSL
                    if eq == 0 and lo == 0:
                        # keep a copy of v_in[h][0, 0:2] to re-write later as
                        # the v-AllGather's delay dependency
                        nc.vector.tensor_copy(out=vsave[h],
                                              in_=vstg[0:1, 0:2])
                    nc.scalar.dma_start(
                        out=v_in[h][lo:lo + P, eq * NQ:(eq + 1) * NQ],
                        in_=vstg)
                    if eq == EQN - 1 and s == 0:
                        # delayed kt AG 3: its HBM burst lands in Q-proj's
                        # stream instead of V-proj's evacuation bursts
                        kz = res.tile([1, 2], bf16, name="kz")
                        nc.vector.tensor_scalar_mul(kz, vstg[0:1, 0:2], 0.0)
                        nc.vector.tensor_tensor(out=kz, in0=kz, in1=ksb,
                                                op=mybir.AluOpType.add)
                        nc.scalar.dma_start(out=kt_in[KAG - 1][0:1, 0:2],
                                            in_=kz)
                        nc.gpsimd.collective_compute(
                            "AllGather", mybir.AluOpType.bypass,
                            replica_groups=groups,
                            ins=[kt_in[KAG - 1].opt()],
                            outs=[kt_all[KAG - 1].opt()])

            # --- phase 0c: QT_i = Wq^T @ xT_i + bq -> qt_sb (SBUF-resident) -
            qk_proj(wq, bq_sb, qt_sb, None, "wq", 0, ECH)

            # --- phase 1: ST[j] = KT_j^T @ QT, exp; l_acc on Vector engine --
            st_sb = big.tile([P, JN, SL], bf16, tag="bigslot")
            l_acc = res.tile([P, SL], f32, tag="lacc")
            nc.vector.memset(l_acc, 0.0)
            ec_per_slice = ECH // KAG  # 4 e-chunks per AG slice
            for j in range(JN):
                r, c = j // SCH, j % SCH
                # shares the "w" tag: its buffer slot only frees late in
                # phase 0, which pins this DMA's static queue position past
                # the AllGathers (a hoisted AG-dependent DMA head-of-line
                # blocks the whole sync queue).
                kt_t = wstr.tile([P, ECH, P], bf16, tag="w", name=f"kt_{j}")
                for i in range(KAG):
                    nc.sync.dma_start(
                        out=kt_t[:, i * ec_per_slice:(i + 1) * ec_per_slice, :],
                        in_=kt_all[i][r * KSL:(r + 1) * KSL,
                                      c * P:(c + 1) * P].rearrange(
                                          "(ec p) s -> p ec s", p=P))
                for q in range(QCH):
                    st_ps = ps.tile([P, NQ], f32, tag="mm", name=f"st_{j}_{q}")
                    for ec in range(ECH):
                        nc.tensor.matmul(
                            st_ps, kt_t[:, ec],
                            qt_sb[:, ec, q * NQ:(q + 1) * NQ],
                            start=(ec == 0), stop=(ec == ECH - 1))
                    nc.scalar.activation(
                        st_sb[:, j, q * NQ:(q + 1) * NQ], st_ps,
                        mybir.ActivationFunctionType.Exp, scale=SCALE)
                # running softmax denominators on the idle Vector engine
                nc.vector.tensor_tensor(
                    out=l_acc, in0=l_acc, in1=st_sb[:, j, :],
                    op=mybir.AluOpType.add)
                # v AllGathers fire from inside phase 1, where model DMA
                # demand is low — phase 0 then only carries the kt-AG HBM
                # bursts. The AG's input is made to genuinely depend on
                # st_sb[., j, .]: re-write 2 bytes of v_in with identical
                # values computed as 0*st + vsave (exact).
                if j == 12 or j == 36:
                    h = 0 if j == 12 else 1
                    vz = res.tile([1, 2], bf16, name=f"vz_{h}")
                    nc.vector.tensor_scalar_mul(vz, st_sb[0:1, j, 0:2], 0.0)
                    nc.vector.tensor_tensor(out=vz, in0=vz, in1=vsave[h],
                                            op=mybir.AluOpType.add)
                    nc.scalar.dma_start(out=v_in[h][0:1, 0:2], in_=vz)
                    nc.gpsimd.collective_compute(
                        "AllGather", mybir.AluOpType.bypass,
                        replica_groups=groups,
                        ins=[v_in[h].opt()],
                        outs=[v_all[h].opt()])

            # --- phase 1b: partition-reduce l_acc, reciprocal ---------------
            l_ps = [ps.tile([1, NQ], f32, tag="mm", name=f"l_{q}")
                    for q in range(QCH)]
            for q in range(QCH):
                nc.tensor.matmul(l_ps[q], ones_sb,
                                 l_acc[:, q * NQ:(q + 1) * NQ],
                                 start=True, stop=True)
            l_row = res.tile([1, SL], f32, tag="lacc")
            for q in range(QCH):
                nc.vector.tensor_copy(out=l_row[:, q * NQ:(q + 1) * NQ],
                                      in_=l_ps[q])
            nc.scalar.dma_start(out=l_dram, in_=l_row)
            l_pp = res.tile([P, SCH], f32)
            nc.sync.dma_start(out=l_pp,
                              in_=l_dram[0].rearrange("(c p) -> p c", p=P))
            recip = res.tile([P, SCH], f32)
            nc.vector.reciprocal(recip, l_pp)
            _bv_ap = bv.ap()

            # --- phase 2: O = exp(ST)^T @ V, 8 passes of 4 PSUM banks -------
            for eq in range(EQN):
                bv_bcast_ap = bass.AP(tensor=_bv_ap.tensor,
                                      offset=_bv_ap.offset + eq * NQ,
                                      ap=[[0, P], [1, NQ]])
                bv_sb = stg.tile([P, NQ], bf16, tag="bv", name=f"bv_{eq}")
                nc.sync.dma_start(out=bv_sb, in_=bv_bcast_ap)
                for hf in range(2):
                    o_ps = [ps.tile([P, NQ], f32, tag="mm",
                                    name=f"o_{eq}_{hf}_{si}")
                            for si in range(4)]
                    for j in range(JN):
                        r, sloc = j // SCH, (j % SCH) * P
                        h, off = sloc // VSL, sloc % VSL
                        v_t = wstr.tile([P, NQ], bf16, tag="w",
                                        name=f"v_{eq}_{hf}_{j}")
                        nc.sync.dma_start(
                            out=v_t,
                            in_=v_all[h][r * VSL + off:r * VSL + off + P,
                                         eq * NQ:(eq + 1) * NQ])
                        for si in range(4):
                            s = hf * 4 + si
                            nc.tensor.matmul(
                                o_ps[si], st_sb[:, j, s * P:(s + 1) * P], v_t,
                                start=(j == 0), stop=(j == JN - 1))
                    for si in range(4):
                        s = hf * 4 + si
                        o_stg = stg.tile([P, NQ], f32, tag="stg",
                                         name=f"ostg_{eq}_{hf}_{si}")
                        nc.vector.tensor_scalar_mul(o_stg, o_ps[si],
                                                    recip[:, s:s + 1])
                        nc.vector.tensor_tensor(
                            out=o_stg, in0=o_stg, in1=bv_sb,
                            op=mybir.AluOpType.add)
                        nc.scalar.dma_start(
                            out=out[s * P:(s + 1) * P, eq * NQ:(eq + 1) * NQ],
                            in_=o_stg)

    nc.compile()
    return nc


def kernel(x, Wq, bq, Wk, bk, Wv, bv):
    from concourse.bass_utils import run_bass_kernel_spmd

    xt = np.ascontiguousarray(x.astype(_BF16).T)          # [E, S] bf16

    def _pre(w):  # [e_in, e_out] -> [eo, p, c, n] so each eo-slice is contiguous
        return np.ascontiguousarray(
            w.astype(_BF16).reshape(ECH, P, ECH, P).transpose(2, 1, 0, 3))

    wqb = _pre(Wq)
    wkb = _pre(Wk)
    wvb = np.ascontiguousarray(Wv.astype(_BF16))
    bqf = np.ascontiguousarray(bq.astype(np.float32))
    bkf = np.ascontiguousarray(bk.astype(np.float32))
    bvf = np.ascontiguousarray(bv.astype(_BF16))

    in_maps = []
    for r in range(N_CORES):
        in_maps.append({
            "xt": np.ascontiguousarray(xt[:, r * SL:(r + 1) * SL]),
            "wq": wqb, "wk": wkb, "wv": wvb,
            "bq": bqf, "bk": bkf, "bv": bvf,
        })

    nc = _build()
    res = run_bass_kernel_spmd(nc, in_maps, core_ids=list(range(N_CORES)))
    global LAST_RESULT
    LAST_RESULT = res
    return np.concatenate([res.results[r]["out"] for r in range(N_CORES)],
                          axis=0).astype(np.float32)


LAST_RESULT = None
